# revision 36
# baseline (speedup 1.0000x reference)
"""Local 3x3 attention on 8 TRN2 NeuronCores — hand-written Bass/Tile kernel.

Problem: q,k,v [32, 128, 64, 64] f32; per pixel, attend over the 3x3
neighborhood (zero-padded) with softmax over the 9 logits; out [32,64,64,128].

Sharding: pure data-parallel, 4 images per core across 8 cores.

Device algorithm (per image, [128 d, 4096 pix] layout, pixel tiles of 128 =
two image rows):
  For window tile t (rows 2t, 2t+1) the 3x3 halo spans image rows
  2t-1 .. 2t+2.  Scores are computed TRANSPOSED via PE matmuls
  (contract over d): scoresT[halo_pix, pix] = k_slice.T @ q_tile, as one
  "pair" chunk (rows 2t,2t+1 -> [128,128]) plus two "single" chunks
  (rows 2t-1 and 2t+2 -> [64,128]).  ScalarE applies exp(scale*x) straight
  out of PSUM into fp16 SBUF; a 0/1 band mask zeroes the non-neighbor
  entries; PE then contracts attnT.T @ vT over the halo into PSUM [128,129]
  where vT carries an extra all-ones column so column 128 accumulates the
  softmax denominator for free.  Out-of-image neighbors contribute
  exp(0)=1 to the reference denominator; a per-pixel constant corr tile
  adds that count.  VectorE computes recip(denom+corr) and scales.
  v arrives pre-transposed from the host (the [pix,d] permute rides the
  f32->fp16 cast pass), so no transposes run on the device at all.

Host ships q,k,v as fp16 (halves transfer/HBM traffic; PE fp16 matmul is
4x faster than fp32); the device returns fp16 in a [p,g,d]-blocked layout
(1 KB DMA runs) that the host inverts during the f32 upcast.
"""

import numpy as np
from contextlib import ExitStack

import concourse.bass as bass
import concourse.tile as tile
from concourse import mybir

B, D, H, W = 32, 128, 64, 64
NCORES = 8
BL = B // NCORES          # images per core
NPIX = H * W              # 4096
NT = H // 2               # 32 two-row window tiles per image
SCALE = float(D) ** -0.5
F16 = mybir.dt.float16
F32 = mybir.dt.float32
EXP = mybir.ActivationFunctionType.Exp


def _host_consts():
    c = np.arange(128) % 64            # pixel column within its row
    hc = np.arange(64)                 # halo column
    band = (np.abs(hc[:, None] - c[None, :]) <= 1).astype(np.float16)  # [64,128]
    m_pair = np.concatenate([band, band], axis=0)                      # [128,128]
    m_first = band * (np.arange(128)[None, :] < 64)    # halo row 2t-1: r=0 only
    m_last = band * (np.arange(128)[None, :] >= 64)    # halo row 2t+2: r=1 only
    # single combined mask [128,256]: cols 0:128 = pair chunk, cols 128:256 =
    # singles chunk (partitions 0:64 = m_last at base 0, 64:128 = m_first at
    # base 64, matching the vT half-slices used in the AV matmuls).
    m_sing = np.concatenate([m_last, m_first], axis=0)
    m_all = np.concatenate([m_pair, m_sing] * 4, axis=1).astype(np.float16)
    # corr[p, t] = number of out-of-image neighbors for pixel p of tile t
    # (each contributes exp(0)=1 to the reference softmax denominator).
    r = np.arange(128) // 64
    vc = np.where((c == 0) | (c == 63), 2, 3)
    corr = np.zeros((128, NT), np.float32)
    for t in range(NT):
        vd = np.full(128, 3)
        if t == 0:
            vd = np.where(r == 0, 2, 3)
        if t == NT - 1:
            vd = np.where(r == 1, 2, 3)
        corr[:, t] = 9 - vd * vc
    return m_all, corr


def _body(ctx, tc, qd, kd, vd, mp, cr, od, sim=False):
    nc = tc.nc

    # vT: host ships v pre-transposed as [128 pix-in-tile, NT*130] per image
    # (col 128 of each 130 block = 1.0 for the denominator trick, col 129
    # pad).  One DMA per image, double-buffered by image parity.
    vt_pool = ctx.enter_context(tc.tile_pool(name="vt", bufs=1))
    vt_all = [vt_pool.tile([128, NT * 130], F16, tag=f"vtall_{s}",
                           name=f"vtall_{s}") for s in range(2)]

    qk_pool = ctx.enter_context(tc.tile_pool(name="qk", bufs=3))
    consts = ctx.enter_context(tc.tile_pool(name="consts", bufs=1))
    ps_sc = ctx.enter_context(tc.tile_pool(name="ps_sc", bufs=2, space="PSUM"))
    ps_av = ctx.enter_context(tc.tile_pool(name="ps_av", bufs=4, space="PSUM"))
    at_pool = ctx.enter_context(tc.tile_pool(name="at", bufs=3))
    sm_pool = ctx.enter_context(tc.tile_pool(name="sm", bufs=4))
    out_pool = ctx.enter_context(tc.tile_pool(name="outp", bufs=2))

    m_all4 = None
    corr = None
    for i in range(BL):
        s = i % 2
        q_t = qk_pool.tile([128, NPIX], F16, tag="q")
        k_t = qk_pool.tile([128, NPIX], F16, tag="k")
        if i == 0:
            # split image-0 loads so the first tiles' data lands sooner
            nc.sync.dma_start(k_t[:, 0:1024], kd[i][:, 0:1024])
            nc.sync.dma_start(q_t[:, 0:1024], qd[i][:, 0:1024])
            nc.scalar.dma_start(vt_all[s][:], vd[i])
            nc.sync.dma_start(k_t[:, 1024:], kd[i][:, 1024:])
            nc.sync.dma_start(q_t[:, 1024:], qd[i][:, 1024:])
            # consts land after the image-0 loads so they don't delay them
            m_all4 = consts.tile([128, 1024], F16, tag="mall")
            nc.sync.dma_start(m_all4[:], mp[:])
            corr = consts.tile([128, NT], F32, tag="corr")
            nc.sync.dma_start(corr[:], cr[:])
        else:
            nc.sync.dma_start(q_t[:], qd[i])
            nc.sync.dma_start(k_t[:], kd[i])
            nc.scalar.dma_start(vt_all[s][:], vd[i])
        # one iteration per QUAD of window tiles for scores/exp/mask;
        # AV + epilogue run per pair inside.
        for w in range(NT // 4):
            ts4 = [4 * w + j for j in range(4)]
            # scoresT for 4 tiles in two [128,512] PSUM tiles (1 bank each);
            # per-tile 256-block: [pair | singles(s3@base0, s0@base64)]
            sc_a = ps_sc.tile([128, 512], F32, tag="sca")
            sc_b = ps_sc.tile([128, 512], F32, tag="scb")
            for h, t in enumerate(ts4):
                sc = sc_a if h < 2 else sc_b
                qs = q_t[:, 128 * t:128 * (t + 1)]
                o = 256 * (h % 2)
                nc.tensor.matmul(sc[:, o:o + 128],
                                 lhsT=k_t[:, 128 * t:128 * (t + 1)],
                                 rhs=qs, start=True, stop=True)
                if t < NT - 1:
                    nc.tensor.matmul(sc[0:64, o + 128:o + 256],
                                     lhsT=k_t[:, 64 * (2 * t + 2):64 * (2 * t + 3)],
                                     rhs=qs, start=True, stop=True)
                elif sim:
                    nc.vector.memset(sc[0:64, o + 128:o + 256], 0.0)
                if t > 0:
                    nc.tensor.matmul(sc[64:128, o + 128:o + 256],
                                     lhsT=k_t[:, 64 * (2 * t - 1):64 * 2 * t],
                                     rhs=qs, start=True, stop=True)
                elif sim:
                    nc.vector.memset(sc[64:128, o + 128:o + 256], 0.0)
            # one exp (ScalarE) + one mask (GpSimd) for the quad
            at = at_pool.tile([128, 1024], F16, tag="at")
            nc.scalar.activation(at[:, 0:512], sc_a[:], EXP, scale=SCALE)
            nc.scalar.activation(at[:, 512:1024], sc_b[:], EXP, scale=SCALE)
            nc.vector.tensor_mul(at[:], at[:], m_all4[:])
            # output staging for the quad
            o_t = out_pool.tile([128, 512], F16, tag="o")
            for g in range(2):  # two pairs in the quad
                t0 = ts4[2 * g]
                av2 = ps_av.tile([128, 264], F32, tag="av2")
                for h2 in range(2):
                    t = t0 + h2
                    o = 256 * (2 * g + h2)
                    avs = av2[:, 132 * h2:132 * h2 + 129]
                    mm = []
                    if t > 0:
                        mm.append((at[64:128, o + 128:o + 256],
                                   vt_all[s][64:128,
                                             130 * (t - 1):130 * (t - 1) + 129]))
                    mm.append((at[:, o:o + 128],
                               vt_all[s][:, 130 * t:130 * t + 129]))
                    if t < NT - 1:
                        mm.append((at[0:64, o + 128:o + 256],
                                   vt_all[s][0:64,
                                             130 * (t + 1):130 * (t + 1) + 129]))
                    for j, (a, vv) in enumerate(mm):
                        nc.tensor.matmul(avs, lhsT=a, rhs=vv,
                                         start=(j == 0), stop=(j == len(mm) - 1))
                    dnm = sm_pool.tile([128, 1], F32, tag="dnm")
                    nc.vector.tensor_add(dnm[:],
                                         av2[:, 132 * h2 + 128:132 * h2 + 129],
                                         corr[:, t:t + 1])
                    rec = sm_pool.tile([128, 1], F32, tag="rec")
                    nc.vector.reciprocal(rec[:], dnm[:])
                    gslot = 2 * g + h2
                    nc.vector.tensor_scalar_mul(
                        o_t[:, 128 * gslot:128 * (gslot + 1)],
                        av2[:, 132 * h2:132 * h2 + 128], rec[:])
            # store the quad: DRAM block laid out [p, g, d] (1KB runs);
            # host inverts the (p, g) swap during the f32 upcast pass.
            dd = od[i, 512 * w:512 * (w + 1), :].rearrange(
                '(p g) d -> p g d', p=128, g=4)
            nc.sync.dma_start(dd, o_t[:].rearrange('p (g x) -> p g x', g=4))


def _legalize_dma_waits(nc):
    """walrus's PSEUDO_DMA_DIRECT2D codegen accepts at most one sync-wait
    per DMA instruction, but Tile emits 2-3 (own-lane ring wait + real
    deps).  Hoist every DMA wait onto same-engine NoOps directly before the
    DMA — engine program order makes this equivalent."""
    keep = (mybir.InstUnconditionalBranch, mybir.InstCompareAndBranch,
            mybir.InstHalt)
    f = nc.m.functions[0]
    for bb in f.blocks:
        new = []
        changed = False
        for inst in bb.instructions:
            si = inst.sync_info
            if (si is not None and len(si.on_wait) > 1
                    and not isinstance(inst, keep)):
                for wi, w in enumerate(si.on_wait[1:]):
                    new.append(mybir.InstNoOp(
                        name=f"{inst.name}-w{wi}",
                        engine=inst.engine,
                        sync_info=mybir.SyncInfo(on_wait=[w], on_update=[]),
                        bass_nofuse=True,
                    ))
                inst.sync_info = mybir.SyncInfo(
                    on_wait=[si.on_wait[0]], on_update=list(si.on_update))
                changed = True
            new.append(inst)
        if changed:
            bb.instructions = new


def _build(legalize=True, sim=False):
    nc = bass.Bass()
    qd = nc.declare_dram_parameter("q", [BL, D, NPIX], F16, isOutput=False)
    kd = nc.declare_dram_parameter("k", [BL, D, NPIX], F16, isOutput=False)
    vd = nc.declare_dram_parameter("v", [BL, 128, NT * 130], F16, isOutput=False)
    mp = nc.declare_dram_parameter("mall", [128, 1024], F16, isOutput=False)
    cr = nc.declare_dram_parameter("corr", [128, NT], F32, isOutput=False)
    od = nc.declare_dram_parameter("out", [BL, NPIX, D], F16, isOutput=True)
    with tile.TileContext(nc) as tc:
        with ExitStack() as ctx:
            _body(ctx, tc, qd, kd, vd, mp, cr, od, sim=sim)
    if legalize:
        _legalize_dma_waits(nc)
    return nc


_CACHE = {}


def _get_nc():
    if "nc" not in _CACHE:
        _CACHE["nc"] = _build()
    return _CACHE["nc"]


def _get_runner():
    """Cached PJRT runner (same mechanics as bass2jax.run_bass_via_pjrt, but
    the jitted callable is built once so repeat kernel() calls don't re-trace
    or re-ship output donation buffers from the host)."""
    if "run" in _CACHE:
        return _CACHE["run"]
    import jax
    import jax.numpy as jnp
    from jax.sharding import Mesh, PartitionSpec, NamedSharding
    from jax.experimental.shard_map import shard_map
    from concourse import bass2jax, mybir as mb

    nc = _get_nc()
    bass2jax.install_neuronx_cc_hook()

    part_name = nc.partition_id_tensor.name if nc.partition_id_tensor else None
    in_names, out_names, out_avals = [], [], []
    for alloc in nc.m.functions[0].allocations:
        if not isinstance(alloc, mb.MemoryLocationSet):
            continue
        name = alloc.memorylocations[0].name
        if alloc.kind == "ExternalInput":
            if name != part_name:
                in_names.append(name)
        elif alloc.kind == "ExternalOutput":
            out_names.append(name)
            out_avals.append(jax.core.ShapedArray(
                tuple(alloc.tensor_shape), mb.dt.np(alloc.dtype)))
    n_params = len(in_names)
    all_in_names = in_names + out_names
    if part_name is not None:
        all_in_names = all_in_names + [part_name]

    def _body_fn(*args):
        operands = list(args)
        if part_name is not None:
            operands.append(bass2jax.partition_id_tensor())
        outs = bass2jax._bass_exec_p.bind(
            *operands,
            out_avals=tuple(out_avals),
            in_names=tuple(all_in_names),
            out_names=tuple(out_names),
            lowering_input_output_aliases=(),
            sim_require_finite=True,
            sim_require_nnan=True,
            nc=nc,
        )
        return tuple(outs)

    devices = jax.devices()[:NCORES]
    mesh = Mesh(np.asarray(devices), ("core",))
    n_outs = len(out_names)
    in_specs = (PartitionSpec("core"),) * (n_params + n_outs)
    out_specs = (PartitionSpec("core"),) * n_outs
    sharded = jax.jit(
        shard_map(_body_fn, mesh=mesh, in_specs=in_specs, out_specs=out_specs,
                  check_rep=False),
        donate_argnums=tuple(range(n_params, n_params + n_outs)),
        keep_unused=True,
    )
    zero_sharding = NamedSharding(mesh, PartitionSpec("core"))
    zero_fns = [
        jax.jit(
            (lambda shp, dt: (lambda: jnp.zeros(shp, dt)))(
                (NCORES * av.shape[0], *av.shape[1:]), av.dtype),
            out_shardings=zero_sharding)
        for av in out_avals
    ]

    def run(global_inputs):
        # global_inputs: dict name -> np array with per-core axis-0 concat
        zeros = [zf() for zf in zero_fns]
        outs = sharded(*[global_inputs[n] for n in in_names], *zeros)
        return {name: outs[i] for i, name in enumerate(out_names)}

    _CACHE["run"] = run
    return run


_M_ALL, _CORR = _host_consts()
_M_ALL_G = np.tile(_M_ALL, (NCORES, 1))
_CORR_G = np.tile(_CORR, (NCORES, 1))

# Preallocated host buffer for the pre-transposed v: [B, 128 pix-in-tile,
# NT tiles, 130], col 128 = 1.0 (denominator ones column), col 129 = 0 pad.
_VT_BUF = np.zeros((B, 128, NT, 130), np.float16)
_VT_BUF[..., 128] = 1.0


def _prep_v(v):
    # v: [B, D, H, W] f32 -> [B, 128 pix, NT, 128 d] fp16 into _VT_BUF
    _VT_BUF[..., :128] = v.reshape(B, D, NT, 128).transpose(0, 3, 2, 1)
    return _VT_BUF.reshape(B, 128, NT * 130)


def kernel(q, k, v):
    qh = np.asarray(q, dtype=np.float16).reshape(B, D, NPIX)
    kh = np.asarray(k, dtype=np.float16).reshape(B, D, NPIX)
    vh = _prep_v(np.asarray(v).reshape(B, D, H, W))
    run = _get_runner()
    outs = run({
        "q": qh, "k": kh, "v": vh,
        "mall": _M_ALL_G, "corr": _CORR_G,
    })
    out = np.asarray(outs["out"])  # [B, NPIX, D] fp16, blocks of [p,g,d]
    out = out.reshape(B, NT // 4, 128, 4, D).transpose(0, 1, 3, 2, 4)
    return out.astype(np.float32).reshape(B, H, W, D)


def _profile(inputs):
    """Run once under the axon NTFF profiling hook; returns max-core HW
    exec time in ns (None if the hook is unavailable). For test.py only."""
    from concourse.bass_utils import run_bass_kernel_spmd
    nc = _get_nc()
    qh = np.asarray(inputs["q"], dtype=np.float16).reshape(B, D, NPIX)
    kh = np.asarray(inputs["k"], dtype=np.float16).reshape(B, D, NPIX)
    vh = _prep_v(np.asarray(inputs["v"]).reshape(B, D, H, W))
    in_maps = []
    for c in range(NCORES):
        in_maps.append({
            "q": qh[c * BL:(c + 1) * BL],
            "k": kh[c * BL:(c + 1) * BL],
            "v": vh[c * BL:(c + 1) * BL],
            "mall": _M_ALL, "corr": _CORR,
        })
    br = run_bass_kernel_spmd(nc, in_maps, core_ids=list(range(NCORES)),
                              trace=True)
    res = {
        "exec_time_ns": br.exec_time_ns,
        "mean_exec_time_ns": br.mean_exec_time_ns,
        "trace": br.instructions_and_trace,
    }
    _CACHE["last_profile"] = res
    if br.instructions_and_trace is not None:
        print("trace path:", br.instructions_and_trace[1])
    return br.exec_time_ns
